# revision 1
# baseline (speedup 1.0000x reference)
"""nn_AttnDecoderCell — Trainium2 Bass kernel (8 NeuronCores, data-parallel).

kernel(**inputs) takes the FULL unsharded inputs (as produced by
setup_inputs(): x[512,1024], state[512,1024], constants[512,256,1024],
w_att[2048,1], b_att[1], w_z/u_z/b_z, w_r/u_r/b_r, w_h/u_h/b_h) and returns
the full s_t [512, 1024] float32.

Sharding: batch dim split 64 rows per core; weights replicated.

Per-core program (all f32):
 - Attention: C streamed as [l(128p), d] tiles; energy = DVE tensor_tensor
   mult against a broadcast w_att_c row + ACT Copy-with-accum reduction over
   d (softmax shift-invariance makes the state@w_s + b_att terms irrelevant);
   exp on ACT (|energy| small enough that no max-subtraction is needed);
   v rows accumulated in PSUM with the exp column [128,1] as the (cheap)
   stationary and C chunks [128,512] as the moving operand (f32 stationary
   loads of [128,128] cost ~820 cyc, so the reversed orientation was 1.6x
   slower); PSUM rows staged to a partition-0 SBUF row on ACT and scattered
   to their batch partition by tiny SBUF->SBUF DMAs (compute engines cannot
   address SBUF at non-32-aligned base partitions; DMA is exempt); vT for the
   GRU built by PE transposes; sumexp via 2 batched ones-matmuls; normalize
   by 1/sumexp at the end.
 - GRU: f32 matmuls with batch on PSUM partitions; state.T/x.T/(r*state).T
   built via PE transposes; weights streamed from DRAM as the moving operand;
   bias added with a rank-1 ones matmul into the same PSUM accumulation
   group; sigmoid/tanh read straight from PSUM on ACT; final combine
   s_t = h + z*(state-h) + v on DVE.
"""

from contextlib import ExitStack

import numpy as np

import concourse.bacc as bacc
import concourse.bass as bass
import concourse.tile as tile
import concourse.mybir as mybir
from concourse.bass_utils import run_bass_kernel_spmd
from concourse.masks import make_identity

f32 = mybir.dt.float32
AF = mybir.ActivationFunctionType
ALU = mybir.AluOpType

B, L, D, DIN = 512, 256, 1024, 1024
N_CORES = 8
Bc = B // N_CORES          # 64 batch rows per core
LT = L // 128              # 2 l-tiles
KW = D // 128              # 8 k-tiles for W matmuls
KU = (DIN + D) // 128      # 16 k-tiles for U matmuls
NCH = D // 512             # 2 psum chunks of 512 output cols


def _build(loop_n=1, G=2, c_bufs=3, w_bufs=4, u_bufs=4):
    nc = bacc.Bacc("TRN2", target_bir_lowering=False, debug=False,
                   num_devices=N_CORES)
    x_d = nc.dram_tensor("x", [Bc, DIN], f32, kind="ExternalInput").ap()
    s_d = nc.dram_tensor("state", [Bc, D], f32, kind="ExternalInput").ap()
    c_d = nc.dram_tensor("constants", [Bc, L, D], f32, kind="ExternalInput").ap()
    watt_d = nc.dram_tensor("w_att", [2 * D, 1], f32, kind="ExternalInput").ap()
    w_g, u_g, b_g = {}, {}, {}
    for g in "zrh":
        w_g[g] = nc.dram_tensor(f"w_{g}", [D, D], f32, kind="ExternalInput").ap()
        u_g[g] = nc.dram_tensor(f"u_{g}", [DIN + D, D], f32,
                                kind="ExternalInput").ap()
        b_g[g] = nc.dram_tensor(f"b_{g}", [D], f32, kind="ExternalInput").ap()
    o_d = nc.dram_tensor("out", [Bc, D], f32, kind="ExternalOutput").ap()

    with tile.TileContext(nc) as tc:
      def body(_i):
        es = ExitStack()
        small = es.enter_context(tc.tile_pool(name="small", bufs=1))
        cpool = es.enter_context(tc.tile_pool(name="cpool", bufs=c_bufs))
        scr = es.enter_context(tc.tile_pool(name="scr", bufs=2))
        wpool = es.enter_context(tc.tile_pool(name="wpool", bufs=w_bufs))
        upool = es.enter_context(tc.tile_pool(name="upool", bufs=u_bufs))
        psT = es.enter_context(tc.tile_pool(name="psT", bufs=1, space="PSUM"))
        psA = es.enter_context(tc.tile_pool(name="psA", bufs=1, space="PSUM"))
        psV = es.enter_context(tc.tile_pool(name="psV", bufs=2, space="PSUM"))
        vst = es.enter_context(tc.tile_pool(name="vst", bufs=3))
        psG = es.enter_context(tc.tile_pool(name="psG", bufs=4, space="PSUM"))

        # ---------------- setup ----------------
        ident = small.tile([128, 128], f32)
        make_identity(nc, ident[:])
        wc_rep = small.tile([128, D], f32)
        nc.sync.dma_start(
            wc_rep[:],
            bass.AP(tensor=watt_d.tensor, offset=D, ap=[[0, 128], [1, D]]))
        ones_col = small.tile([128, 1], f32)
        nc.vector.memset(ones_col[:], 1.0)
        ones_row = small.tile([1, Bc], f32)
        nc.vector.memset(ones_row[:], 1.0)
        brow = {}
        for g in "zrh":
            brow[g] = small.tile([1, D], f32, name=f"brow_{g}")
            nc.sync.dma_start(
                brow[g][:],
                bass.AP(tensor=b_g[g].tensor, offset=0, ap=[[0, 1], [1, D]]))

        xs = small.tile([Bc, DIN], f32)
        nc.sync.dma_start(xs[:], x_d[:])
        ss = small.tile([Bc, D], f32)
        nc.sync.dma_start(ss[:], s_d[:])

        def transpose_to(dst3, src2d):
            n = dst3.shape[1]
            for ch in range(n):
                tp = psT.tile([128, Bc], f32, name="tp", tag="tp")
                nc.tensor.transpose(tp[:], src2d[:, ch * 128:(ch + 1) * 128],
                                    ident[:Bc, :Bc])
                nc.vector.tensor_copy(out=dst3[:, ch, :], in_=tp[:])

        sT = small.tile([128, KW, Bc], f32)
        transpose_to(sT, ss)
        xT = small.tile([128, KW, Bc], f32)
        transpose_to(xT, xs)

        # ---------------- attention ----------------
        eT = small.tile([128, LT, Bc], f32)
        expT = small.tile([128, LT, Bc], f32)
        s_ps = psA.tile([1, Bc], f32)
        v_un = small.tile([Bc, D], f32)

        for gi in range(Bc // G):
            b0 = gi * G
            ct = cpool.tile([128, G, LT, D], f32, tag="ct")
            nc.sync.dma_start(
                ct[:],
                c_d[b0:b0 + G].rearrange("b (t p) d -> p b t d", p=128))
            for bi in range(G):
                for lt in range(LT):
                    prod = scr.tile([128, D], f32, tag="prod")
                    eng = nc.vector if (bi * LT + lt) % 2 == 0 else nc.gpsimd
                    eng.tensor_tensor(out=prod[:], in0=ct[:, bi, lt, :],
                                      in1=wc_rep[:], op=ALU.mult)
                    nc.scalar.activation(
                        out=prod[:], in_=prod[:], func=AF.Copy,
                        accum_out=eT[:, lt, b0 + bi:b0 + bi + 1])
            nc.scalar.activation(out=expT[:, :, b0:b0 + G],
                                 in_=eT[:, :, b0:b0 + G], func=AF.Exp)
            for bi in range(G):
                b = b0 + bi
                stage = vst.tile([1, D], f32, tag="vstage")
                for chn in range(2):
                    vp = psV.tile([1, 512], f32, name="vp", tag="vp")
                    for lt in range(LT):
                        nc.tensor.matmul(
                            vp[:], expT[:, lt, b:b + 1],
                            ct[:, bi, lt, chn * 512:(chn + 1) * 512],
                            start=(lt == 0), stop=(lt == LT - 1),
                            skip_group_check=True)
                    nc.scalar.copy(stage[0:1, chn * 512:(chn + 1) * 512],
                                   vp[:])
                nc.sync.dma_start(v_un[b:b + 1, :], stage[:])

        for lt in range(LT):
            nc.tensor.matmul(s_ps[:, :], ones_col[:], expT[:, lt, :],
                             start=(lt == 0), stop=(lt == LT - 1),
                             skip_group_check=True)
        recip_row = small.tile([1, Bc], f32)
        nc.vector.reciprocal(recip_row[:], s_ps[:])
        recip_rep = small.tile([128, Bc], f32)
        nc.gpsimd.partition_broadcast(recip_rep[:], recip_row[:])

        vT = small.tile([128, KW, Bc], f32)
        for ch in range(KW):
            tpv = psT.tile([128, Bc], f32, name="tpv", tag="tp")
            nc.tensor.transpose(tpv[:], v_un[:, ch * 128:(ch + 1) * 128],
                                ident[:Bc, :Bc])
            nc.vector.tensor_copy(out=vT[:, ch, :], in_=tpv[:])
        for t in range(2):
            nc.vector.tensor_tensor(
                out=vT[:, 4 * t:4 * t + 4, :], in0=vT[:, 4 * t:4 * t + 4, :],
                in1=recip_rep[:, None, :].broadcast_to([128, 4, Bc]),
                op=ALU.mult)
        rc_ps = psT.tile([Bc, 1], f32, name="rc_ps", tag="tp")
        nc.tensor.transpose(rc_ps[:], recip_row[:, :], ident[:1, :1])
        recip_col = small.tile([Bc, 1], f32)
        nc.vector.tensor_copy(out=recip_col[:], in_=rc_ps[:])
        vs = small.tile([Bc, D], f32)
        nc.vector.tensor_scalar_mul(vs[:], v_un[:], recip_col[:])

        # ---------------- GRU ----------------

        def load_w_tiles(ap, n_ktiles, pool, tag):
            tiles = []
            for t in range(n_ktiles // 2):
                wt = pool.tile([128, 2, D], f32, tag="wt", name=f"{tag}{t}")
                nc.sync.dma_start(
                    wt[:],
                    ap[t * 256:(t + 1) * 256, :].rearrange(
                        "(t p) d -> p t d", p=128))
                tiles.append(wt)
            return tiles

        def gate_psum(g, lhsW3, out_sb, func):
            wt = load_w_tiles(w_g[g], KW, wpool, f"w{g}")
            ut = load_w_tiles(u_g[g], KU, upool, f"u{g}")
            for chn in range(NCH):
                gp = psG.tile([Bc, 512], f32, name="gp", tag="gp")
                for k in range(KW):
                    nc.tensor.matmul(
                        gp[:], lhsW3[:, k, :],
                        wt[k // 2][:, k % 2, chn * 512:(chn + 1) * 512],
                        start=(k == 0), stop=False, skip_group_check=True)
                for k in range(KU):
                    lhs = xT[:, k, :] if k < KW else vT[:, k - KW, :]
                    nc.tensor.matmul(
                        gp[:], lhs,
                        ut[k // 2][:, k % 2, chn * 512:(chn + 1) * 512],
                        start=False, stop=False, skip_group_check=True)
                nc.tensor.matmul(gp[:], ones_row[:],
                                 brow[g][:, chn * 512:(chn + 1) * 512],
                                 start=False, stop=True, skip_group_check=True)
                nc.scalar.activation(out=out_sb[:, chn * 512:(chn + 1) * 512],
                                     in_=gp[:], func=func)

        z_sb = small.tile([Bc, D], f32)
        r_sb = small.tile([Bc, D], f32)
        h_sb = small.tile([Bc, D], f32)
        gate_psum("z", sT, z_sb, AF.Sigmoid)
        gate_psum("r", sT, r_sb, AF.Sigmoid)
        rs_sb = small.tile([Bc, D], f32)
        nc.vector.tensor_tensor(out=rs_sb[:], in0=ss[:], in1=r_sb[:], op=ALU.mult)
        rsT = small.tile([128, KW, Bc], f32)
        transpose_to(rsT, rs_sb)
        gate_psum("h", rsT, h_sb, AF.Tanh)

        d1 = small.tile([Bc, D], f32)
        nc.vector.tensor_tensor(out=d1[:], in0=ss[:], in1=h_sb[:], op=ALU.subtract)
        d2 = small.tile([Bc, D], f32)
        nc.vector.tensor_tensor(out=d2[:], in0=d1[:], in1=z_sb[:], op=ALU.mult)
        d3 = small.tile([Bc, D], f32)
        nc.vector.tensor_tensor(out=d3[:], in0=d2[:], in1=h_sb[:], op=ALU.add)
        o_sb = small.tile([Bc, D], f32)
        nc.vector.tensor_tensor(out=o_sb[:], in0=d3[:], in1=vs[:], op=ALU.add)
        nc.sync.dma_start(o_d[:], o_sb[:])
        es.close()

      if loop_n == 1:
          body(0)
      else:
          with tc.For_i(0, loop_n, 1) as i:
              body(i)

    nc.compile()
    return nc


_NC_CACHE = {}


def _get_nc(loop_n=1):
    if loop_n not in _NC_CACHE:
        _NC_CACHE[loop_n] = _build(loop_n=loop_n)
    return _NC_CACHE[loop_n]


def _in_maps(inputs):
    maps = []
    for c in range(N_CORES):
        lo, hi = c * Bc, (c + 1) * Bc
        m = {
            "x": np.ascontiguousarray(np.asarray(inputs["x"], np.float32)[lo:hi]),
            "state": np.ascontiguousarray(
                np.asarray(inputs["state"], np.float32)[lo:hi]),
            "constants": np.ascontiguousarray(
                np.asarray(inputs["constants"], np.float32)[lo:hi]),
            "w_att": np.asarray(inputs["w_att"], np.float32),
        }
        for g in "zrh":
            m[f"w_{g}"] = np.asarray(inputs[f"w_{g}"], np.float32)
            m[f"u_{g}"] = np.asarray(inputs[f"u_{g}"], np.float32)
            m[f"b_{g}"] = np.asarray(inputs[f"b_{g}"], np.float32)
        maps.append(m)
    return maps


def kernel(**inputs) -> np.ndarray:
    nc = _get_nc(loop_n=1)
    res = run_bass_kernel_spmd(nc, _in_maps(inputs),
                               core_ids=list(range(N_CORES)))
    return np.concatenate([res.results[c]["out"] for c in range(N_CORES)],
                          axis=0).astype(np.float32)



# revision 6
# speedup vs baseline: 2.0099x; 2.0099x over previous
"""nn_AttnDecoderCell — Trainium2 Bass kernel (8 NeuronCores, data-parallel).

kernel(**inputs) takes the FULL unsharded f32 inputs (x[512,1024],
state[512,1024], constants[512,256,1024], w_att[2048,1], b_att[1],
w_z/u_z/b_z, w_r/u_r/b_r, w_h/u_h/b_h) and returns the full s_t
[512, 1024] float32.

Sharding: batch dim split 64 rows per core; weights replicated. All inputs
are cast to bf16 on the host before upload — halves HBM traffic (the kernel
is DMA-bound) and runs the PE at 1 cycle/row instead of f32's 4.

Per-core program:
 - Attention: C streamed as [l(128p), d] bf16 tiles; energy = fused
   multiply+free-dim-reduce (scalar_tensor_tensor with accum_out) against a
   broadcast w_att row, split DVE/Pool (the state@w_att and b_att terms are
   softmax-shift-invariant); exp on ACT; v rows accumulated in PSUM with the
   exp column [128,1] stationary and C [128,512] moving, 4 batches landing
   on psum partitions {0,32,64,96} via col-group tile_position so one ACT
   copy stages 4 rows at once; staged rows PE-transposed into vT[128,8,64]
   (the layout the GRU needs) and transposed back once at the end for the
   [64,1024] row-layout v; sumexp via 2 ones-matmuls; normalization applied
   to vT once after the loop.
 - GRU: bf16 matmuls with batch on PSUM partitions; state.T/x.T/(r*state).T
   built via PE transposes; weights streamed from DRAM bf16 one whole matrix
   per DMA; bias added with a rank-1 ones matmul into the same PSUM group;
   sigmoid/tanh on ACT straight from PSUM; final combine on DVE/Pool.
"""

from contextlib import ExitStack

import numpy as np

import concourse.bacc as bacc
import concourse.bass as bass
import concourse.tile as tile
import concourse.mybir as mybir
from concourse.bass_utils import run_bass_kernel_spmd
from concourse.masks import make_identity

f32 = mybir.dt.float32
bf16 = mybir.dt.bfloat16
BF16_NP = mybir.dt.np(bf16)
AF = mybir.ActivationFunctionType
ALU = mybir.AluOpType

B, L, D, DIN = 512, 256, 1024, 1024
N_CORES = 8
Bc = B // N_CORES          # 64 batch rows per core
LT = L // 128              # 2 l-tiles
KW = D // 128              # 8 k-tiles for W matmuls
KU = (DIN + D) // 128      # 16 k-tiles for U matmuls
G = 4                      # batches per attention group (psum col-groups)
NG = Bc // G               # 16 groups


def _build(loop_n=1, c_bufs=3):
    nc = bacc.Bacc("TRN2", target_bir_lowering=False, debug=False,
                   num_devices=N_CORES)
    x_d = nc.dram_tensor("x", [Bc, DIN], bf16, kind="ExternalInput").ap()
    s_d = nc.dram_tensor("state", [Bc, D], bf16, kind="ExternalInput").ap()
    c_d = nc.dram_tensor("constants", [Bc, L, D], bf16,
                         kind="ExternalInput").ap()
    watt_d = nc.dram_tensor("w_att_c", [1, D], bf16, kind="ExternalInput").ap()
    w_g, u_g, b_g = {}, {}, {}
    for g in "zrh":
        w_g[g] = nc.dram_tensor(f"w_{g}", [D, D], bf16,
                                kind="ExternalInput").ap()
        u_g[g] = nc.dram_tensor(f"u_{g}", [DIN + D, D], bf16,
                                kind="ExternalInput").ap()
        b_g[g] = nc.dram_tensor(f"b_{g}", [1, D], bf16,
                                kind="ExternalInput").ap()
    o_d = nc.dram_tensor("out", [Bc, D], f32, kind="ExternalOutput").ap()

    with tile.TileContext(nc) as tc:
      def body(_i):
        es = ExitStack()
        small = es.enter_context(tc.tile_pool(name="small", bufs=1))
        cpool = es.enter_context(tc.tile_pool(name="cpool", bufs=c_bufs))
        scr = es.enter_context(tc.tile_pool(name="scr", bufs=2))
        st4 = es.enter_context(tc.tile_pool(name="st4", bufs=2))
        wpool = es.enter_context(tc.tile_pool(name="wpool", bufs=2))
        upool = es.enter_context(tc.tile_pool(name="upool", bufs=2))
        psV = es.enter_context(tc.tile_pool(name="psV", bufs=1, space="PSUM"))
        psT = es.enter_context(tc.tile_pool(name="psT", bufs=2, space="PSUM"))
        psG = es.enter_context(tc.tile_pool(name="psG", bufs=4, space="PSUM"))

        # ---------------- setup ----------------
        ident = small.tile([128, 128], bf16)
        make_identity(nc, ident[:])
        wc_rep = small.tile([128, D], bf16)
        nc.sync.dma_start(
            wc_rep[:],
            bass.AP(tensor=watt_d.tensor, offset=0, ap=[[0, 128], [1, D]]))
        ones_col = small.tile([128, 1], bf16)
        nc.vector.memset(ones_col[:], 1.0)
        ones_row = small.tile([1, Bc], bf16)
        nc.vector.memset(ones_row[:], 1.0)
        brow = {}
        for g in "zrh":
            brow[g] = small.tile([1, D], bf16, name=f"brow_{g}")
            nc.sync.dma_start(brow[g][:], b_g[g][:])

        # early weight prefetch for z and r (h streams later, reusing slots)
        def load_w(g):
            wt = wpool.tile([128, KW, D], bf16, tag="wt", name=f"w{g}")
            nc.sync.dma_start(
                wt[:], w_g[g].rearrange("(t p) d -> p t d", p=128))
            ut = upool.tile([128, KU, D], bf16, tag="ut", name=f"u{g}")
            nc.sync.dma_start(
                ut[:], u_g[g].rearrange("(t p) d -> p t d", p=128))
            return wt, ut

        wz, uz = load_w("z")
        wr, ur = load_w("r")

        xs = small.tile([Bc, DIN], bf16)
        nc.sync.dma_start(xs[:], x_d[:])
        ss = small.tile([Bc, D], bf16)
        nc.sync.dma_start(ss[:], s_d[:])

        def transpose_to(dst3, src2d):
            n = dst3.shape[1]
            for ch in range(n):
                tp = psT.tile([128, 8, 128], bf16, name="tp", tag="tp")
                nc.tensor.transpose(tp[:, 0, :Bc],
                                    src2d[:, ch * 128:(ch + 1) * 128],
                                    ident[:Bc, :Bc])
                nc.vector.tensor_copy(out=dst3[:, ch, :], in_=tp[:, 0, :Bc])

        sT = small.tile([128, KW, Bc], bf16)
        transpose_to(sT, ss)
        xT = small.tile([128, KW, Bc], bf16)
        transpose_to(xT, xs)

        # ---------------- attention ----------------
        eT = small.tile([128, LT * Bc], f32)
        expT = small.tile([128, LT * Bc], bf16)
        vT = small.tile([128, KW, Bc], bf16)

        vp = psV.tile([128, 2 * 512], f32)
        nc.vector.memset(vp[:], 0.0)

        for gi in range(NG):
            b0 = gi * G
            ct = cpool.tile([128, G, LT, D], bf16, tag="ct")
            nc.sync.dma_start(
                ct[:],
                c_d[b0:b0 + G].rearrange("b (t p) d -> p b t d", p=128))
            # fused energy: e[l, b] = sum_d C[l, d] * w_att[d]
            for bi in range(G):
                for lt in range(LT):
                    prod = scr.tile([128, D], bf16, tag="prod")
                    col = lt * Bc + b0 + bi
                    if (bi * LT + lt) % 4 == 3:
                        # Pool lacks the fused op: mult there, reduce on ACT
                        nc.gpsimd.tensor_tensor(
                            out=prod[:], in0=ct[:, bi, lt, :],
                            in1=wc_rep[:], op=ALU.mult)
                        nc.scalar.activation(
                            out=prod[:], in_=prod[:], func=AF.Copy,
                            accum_out=eT[:, col:col + 1])
                    else:
                        nc.vector.scalar_tensor_tensor(
                            out=prod[:], in0=ct[:, bi, lt, :], scalar=1.0,
                            in1=wc_rep[:], op0=ALU.bypass, op1=ALU.mult,
                            accum_out=eT[:, col:col + 1])
            for lt in range(LT):
                col = lt * Bc + b0
                nc.scalar.activation(out=expT[:, col:col + G],
                                     in_=eT[:, col:col + G], func=AF.Exp)
            # v rows into psum partitions {0,32,64,96} via col-groups
            for lt in range(LT):
                for chn in range(2):
                    for j in range(G):
                        nc.tensor.matmul(
                            vp[32 * j:32 * j + 1, chn * 512:(chn + 1) * 512],
                            expT[:, lt * Bc + b0 + j:lt * Bc + b0 + j + 1],
                            ct[:, j, lt, chn * 512:(chn + 1) * 512],
                            start=(lt == 0), stop=(lt == LT - 1),
                            skip_group_check=True, tile_position=(0, 32 * j))
            stage4 = st4.tile([128, D], bf16, tag="stage4")
            nc.scalar.copy(stage4[:], vp[:])
            # transpose the 4 staged rows into vT columns
            tp = psT.tile([128, 8, 128], bf16, name="tpv", tag="tp")
            for ch in range(KW):
                nc.tensor.transpose(tp[:, ch, :],
                                    stage4[:, ch * 128:(ch + 1) * 128],
                                    ident[:])
            nc.vector.tensor_copy(out=vT[:, :, b0:b0 + G],
                                  in_=tp[:, :, 0:97:32])

        # sumexp and normalization of vT
        s_ps = psT.tile([1, Bc], f32, name="s_ps", tag="tp")
        for lt in range(LT):
            nc.tensor.matmul(s_ps[:], ones_col[:],
                             expT[:, lt * Bc:(lt + 1) * Bc],
                             start=(lt == 0), stop=(lt == LT - 1),
                             skip_group_check=True)
        recip_row = small.tile([1, Bc], f32)
        nc.vector.reciprocal(recip_row[:], s_ps[:])
        recip_rep = small.tile([128, Bc], f32)
        nc.gpsimd.partition_broadcast(recip_rep[:], recip_row[:])
        for t in range(2):
            nc.vector.tensor_tensor(
                out=vT[:, 4 * t:4 * t + 4, :], in0=vT[:, 4 * t:4 * t + 4, :],
                in1=recip_rep[:, None, :].broadcast_to([128, 4, Bc]),
                op=ALU.mult)
        # transpose normalized vT back to row-layout v
        v_un = small.tile([Bc, D], bf16)
        for ch in range(KW):
            tb = psT.tile([128, 8, 128], bf16, name="tb", tag="tp")
            nc.tensor.transpose(tb[:Bc, 0, :], vT[:, ch, :], ident[:])
            nc.scalar.copy(v_un[:, ch * 128:(ch + 1) * 128], tb[:Bc, 0, :])

        # ---------------- GRU ----------------
        def gate_psum(g, wt, ut, lhsW3, out_sb, func):
            for chn in range(2):
                gp = psG.tile([Bc, 512], f32, name="gp", tag="gp")
                for k in range(KW):
                    nc.tensor.matmul(
                        gp[:], lhsW3[:, k, :],
                        wt[:, k, chn * 512:(chn + 1) * 512],
                        start=(k == 0), stop=False, skip_group_check=True)
                for k in range(KU):
                    lhs = xT[:, k, :] if k < KW else vT[:, k - KW, :]
                    nc.tensor.matmul(
                        gp[:], lhs,
                        ut[:, k, chn * 512:(chn + 1) * 512],
                        start=False, stop=False, skip_group_check=True)
                nc.tensor.matmul(gp[:], ones_row[:],
                                 brow[g][:, chn * 512:(chn + 1) * 512],
                                 start=False, stop=True, skip_group_check=True)
                nc.scalar.activation(out=out_sb[:, chn * 512:(chn + 1) * 512],
                                     in_=gp[:], func=func)

        z_sb = small.tile([Bc, D], bf16)
        r_sb = small.tile([Bc, D], bf16)
        h_sb = small.tile([Bc, D], bf16)
        gate_psum("z", wz, uz, sT, z_sb, AF.Sigmoid)
        gate_psum("r", wr, ur, sT, r_sb, AF.Sigmoid)
        wh, uh = load_w("h")
        rs_sb = small.tile([Bc, D], bf16)
        nc.vector.tensor_tensor(out=rs_sb[:], in0=ss[:], in1=r_sb[:],
                                op=ALU.mult)
        rsT = small.tile([128, KW, Bc], bf16)
        transpose_to(rsT, rs_sb)
        gate_psum("h", wh, uh, rsT, h_sb, AF.Tanh)

        d1 = small.tile([Bc, D], bf16)
        nc.vector.tensor_tensor(out=d1[:], in0=ss[:], in1=h_sb[:],
                                op=ALU.subtract)
        d2 = small.tile([Bc, D], bf16)
        nc.gpsimd.tensor_tensor(out=d2[:], in0=d1[:], in1=z_sb[:],
                                op=ALU.mult)
        d3 = small.tile([Bc, D], bf16)
        nc.vector.tensor_tensor(out=d3[:], in0=d2[:], in1=h_sb[:], op=ALU.add)
        o_sb = small.tile([Bc, D], f32)
        nc.vector.tensor_tensor(out=o_sb[:], in0=d3[:], in1=v_un[:],
                                op=ALU.add)
        nc.sync.dma_start(o_d[:], o_sb[:])
        es.close()

      if loop_n == 1:
          body(0)
      else:
          with tc.For_i(0, loop_n, 1) as i:
              body(i)

    nc.compile()
    return nc


_NC_CACHE = {}


def _get_nc(loop_n=1):
    if loop_n not in _NC_CACHE:
        _NC_CACHE[loop_n] = _build(loop_n=loop_n)
    return _NC_CACHE[loop_n]


def _in_maps(inputs):
    def bf(a):
        return np.ascontiguousarray(np.asarray(a).astype(BF16_NP))

    watt_c = bf(np.asarray(inputs["w_att"], np.float32)[D:2 * D, 0]
                ).reshape(1, D)
    shared = {"w_att_c": watt_c}
    for g in "zrh":
        shared[f"w_{g}"] = bf(inputs[f"w_{g}"])
        shared[f"u_{g}"] = bf(inputs[f"u_{g}"])
        shared[f"b_{g}"] = bf(inputs[f"b_{g}"]).reshape(1, D)
    x_bf = bf(inputs["x"])
    s_bf = bf(inputs["state"])
    c_bf = bf(inputs["constants"])
    maps = []
    for c in range(N_CORES):
        lo, hi = c * Bc, (c + 1) * Bc
        m = dict(shared)
        m["x"] = np.ascontiguousarray(x_bf[lo:hi])
        m["state"] = np.ascontiguousarray(s_bf[lo:hi])
        m["constants"] = np.ascontiguousarray(c_bf[lo:hi])
        maps.append(m)
    return maps


def kernel(**inputs) -> np.ndarray:
    nc = _get_nc(loop_n=1)
    res = run_bass_kernel_spmd(nc, _in_maps(inputs),
                               core_ids=list(range(N_CORES)))
    return np.concatenate([res.results[c]["out"] for c in range(N_CORES)],
                          axis=0).astype(np.float32)
